# revision 15
# baseline (speedup 1.0000x reference)
"""Trainium2 Bass kernel for nn_CostSensitiveCrossEntropyLossN.

Reference semantics (B=131072 samples, C=1000 classes):
    log_probs = log_softmax(outputs)            # [B, C]
    predicted = argmax(outputs, axis=1)         # [B]
    cm = cost_matrix; cm[t_i, p_i] += 1 per sample
    cm = cm * (1 - eye) + 1;  mn = min(cm); mx = max(cm)
    cm = 1 + (cm - mn) / (mx - mn)
    loss = -mean_i(log_probs[i, t_i]) * mean_i(cm[t_i, p_i])

Key identities:
    sum_i cm[t_i, p_i] = sum_{a,b} counts[a,b] * cm[a,b]  (counts = histogram)
    x is streamed in bf16; the winner mask (x == rowmax) may then mark several
    tied columns per row, so the gather mean is normalized by K = sum(counts)
    instead of B (validated: rel err ~2e-4 on the reference data).

Distribution (8 NeuronCores, data-parallel over batch):
  Host deals samples round-robin per 128-class target window, pads each window
  to a uniform tile count so one SPMD program serves all cores, and uploads
  per-tile target one-hots (derived from the integer labels) plus bf16 inputs.

Per 128-sample tile on device:
  ACT: exp(x) with fused row-sum accumulation  -> lse later via Ln
  DVE: row max (half-fold + reduce); winner mask wp = (x == rowmax) in bf16 4x
  PE:  uP[w] += onehot_t^T @ x[:, window]  (target logits)
       counts[w] += onehot_t^T @ wp        (histogram rides the PE)
Tail: per-window ReduceScatter kicked as each window's histogram completes
  (overlaps the main loop), per-core slice of cm normalize, one AllGather of
  6 scalars, final scalar math.
"""
import os
import numpy as np
import ml_dtypes

NCORE = 8
P = 128
C = 1000
NW = 8              # class windows (classes padded to NW*P = 1024)
SR = P // NCORE     # ReduceScatter strip rows per core per window (16)
BETA1, BETA2 = 1.0, 2.0


# ----------------------------------------------------------------------------
# Host-side prep
# ----------------------------------------------------------------------------

def _host_prep(targets):
    t = np.asarray(targets).astype(np.int64)
    tw_all = t // P
    per_cw = [[None] * NW for _ in range(NCORE)]
    for w in range(NW):
        sel = np.where(tw_all == w)[0]
        sel = sel[np.argsort(t[sel], kind="stable")]
        for c in range(NCORE):
            per_cw[c][w] = sel[c::NCORE]
    T_w = []
    for w in range(NW):
        n_max = max(len(per_cw[c][w]) for c in range(NCORE))
        T_w.append(max(1, -(-n_max // P)))
    T = int(sum(T_w))
    rows = np.zeros((NCORE, T * P), dtype=np.int64)
    tloc = np.full((NCORE, T * P), -1, dtype=np.int64)
    valid = np.zeros((NCORE, T * P), dtype=np.float32)
    win_of_tile = np.concatenate(
        [np.full(T_w[w], w, dtype=np.int64) for w in range(NW)])
    for c in range(NCORE):
        off = 0
        for w in range(NW):
            sel = per_cw[c][w]
            n = len(sel)
            cap = T_w[w] * P
            rows[c, off:off + n] = sel
            rows[c, off + n:off + cap] = sel[0] if n > 0 else 0
            tloc[c, off:off + n] = t[sel] - P * w
            valid[c, off:off + n] = 1.0
            off += cap
    return rows, tloc, valid, win_of_tile, T


def _build_inputs(outputs, targets, cost_matrix):
    rows, tloc, valid, win_of_tile, T = _host_prep(targets)
    outputs = np.asarray(outputs, dtype=np.float32)
    cost = np.asarray(cost_matrix, dtype=np.float32)
    ident = np.eye(P, dtype=np.float32)
    mA = np.array([1, 1, 0, 0, 0, 0, 0, 0], dtype=np.float32)[:, None]
    mB = np.array([0, 0, 1, 1, 1, 1, 0, 0], dtype=np.float32)[:, None]
    ar = np.arange(T * P)
    in_maps = []
    for c in range(NCORE):
        x_c = outputs[rows[c]].astype(ml_dtypes.bfloat16)       # [T*P, C]
        ohb = np.zeros((T * P, P), dtype=ml_dtypes.float8_e4m3)
        real = tloc[c] >= 0
        ohb[ar[real], tloc[c][real]] = 1.0
        # this core's post-RS classes: for each window w, rows 16c..16c+15
        cls = (128 * np.repeat(np.arange(NW), SR)
               + SR * c + np.tile(np.arange(SR), NW))            # [P]
        cost_slab = np.full((P, C), 5.0, dtype=np.float32)
        eyec = np.zeros((P, C), dtype=np.float32)
        ok = cls < C
        cost_slab[ok] = cost[cls[ok]]
        eyec[np.arange(P)[ok], cls[ok]] = 1.0
        in_maps.append({
            "x": x_c,
            "ohb": ohb,
            "valid": np.ascontiguousarray(valid[c].reshape(T, P).T),
            "cost": cost_slab,
            "ident": ident,
            "eyec": eyec,
            "eyem": 1.0 - eyec,
            "mA": mA,
            "mB": mB,
        })
    return in_maps, win_of_tile, T


# ----------------------------------------------------------------------------
# Device program
# ----------------------------------------------------------------------------

def _build_program(T, win_of_tile, b_tot, stage=99):
    import concourse.bacc as bacc
    import concourse.tile as tile
    import concourse.mybir as mybir

    f32 = mybir.dt.float32
    bf16 = mybir.dt.bfloat16
    fp8 = mybir.dt.float8e4
    ALU = mybir.AluOpType
    AF = mybir.ActivationFunctionType

    nc = bacc.Bacc("TRN2", target_bir_lowering=False, debug=False,
                   num_devices=NCORE)

    x_d = nc.dram_tensor("x", [T * P, C], bf16, kind="ExternalInput").ap()
    ohb_d = nc.dram_tensor("ohb", [T * P, P], fp8, kind="ExternalInput").ap()
    valid_d = nc.dram_tensor("valid", [P, T], f32, kind="ExternalInput").ap()
    cost_d = nc.dram_tensor("cost", [P, C], f32, kind="ExternalInput").ap()
    ident_d = nc.dram_tensor("ident", [P, P], f32, kind="ExternalInput").ap()
    eyec_d = nc.dram_tensor("eyec", [P, C], f32, kind="ExternalInput").ap()
    eyem_d = nc.dram_tensor("eyem", [P, C], f32, kind="ExternalInput").ap()
    mA_d = nc.dram_tensor("mA", [8, 1], f32, kind="ExternalInput").ap()
    mB_d = nc.dram_tensor("mB", [8, 1], f32, kind="ExternalInput").ap()
    loss_d = nc.dram_tensor("loss", [1, 1], f32, kind="ExternalOutput").ap()

    first = np.zeros(T, dtype=bool)
    last = np.zeros(T, dtype=bool)
    for j in range(T):
        w = win_of_tile[j]
        first[j] = (j == 0) or (win_of_tile[j - 1] != w)
        last[j] = (j == T - 1) or (win_of_tile[j + 1] != w)

    replica = [list(range(NCORE))]
    KB = 8  # tiles per input DMA batch

    with tile.TileContext(nc) as tc:
        with (
            tc.tile_pool(name="io", bufs=1) as io,
            tc.tile_pool(name="xs", bufs=3) as xs,
            tc.tile_pool(name="work", bufs=4) as work,
            tc.tile_pool(name="est", bufs=3) as est,
            tc.tile_pool(name="stg", bufs=2) as stg,
            tc.tile_pool(name="accum", bufs=1) as acc,
            tc.tile_pool(name="ph2", bufs=1) as ph2,
            tc.tile_pool(name="psA", bufs=2, space="PSUM") as psA,
            tc.tile_pool(name="psU", bufs=2, space="PSUM") as psU,
            tc.tile_pool(name="psT", bufs=1, space="PSUM") as psT,
            tc.tile_pool(name="dram", bufs=1, space="DRAM") as dram,
        ):
            # persistent inputs
            valid_sb = io.tile([P, T], f32)
            cost_sb = io.tile([P, C], f32)
            ident_sb = io.tile([P, P], f32)
            eyec_sb = io.tile([P, C], f32)
            eyem_sb = io.tile([P, C], f32)
            mA_sb = io.tile([8, 1], f32)
            mB_sb = io.tile([8, 1], f32)
            for sb, d in ((valid_sb, valid_d), (cost_sb, cost_d),
                          (ident_sb, ident_d), (eyec_sb, eyec_d),
                          (eyem_sb, eyem_d), (mA_sb, mA_d), (mB_sb, mB_d)):
                nc.sync.dma_start(out=sb[:], in_=d)

            # persistent accumulators
            s_sb = acc.tile([P, T], f32)          # row sum(exp)
            lse_sb = acc.tile([P, T], f32)
            u_sb = acc.tile([P, NW, P], f32)
            kacc = acc.tile([P, NW], f32)
            udiag_sb = acc.tile([P, NW], f32)
            nc.vector.memset(u_sb[:], 0.0)

            # per-window collective buffers
            cdram = [dram.tile([P, C], bf16, tag=f"cd{w}", name=f"cd{w}")
                     for w in range(NW)]
            rsout = [dram.tile([SR, C], bf16, tag=f"ro{w}", name=f"ro{w}")
                     for w in range(NW)]

            cA = cB = uP = None
            x2 = oh2 = None
            for j in range(T):
                w = int(win_of_tile[j])
                wlo = w * P
                whi = min(C, wlo + P)
                ncls = whi - wlo

                if j % KB == 0:
                    kk = min(KB, T - j)
                    x2 = xs.tile([P, KB, C], bf16, tag="x")
                    oh2 = xs.tile([P, KB, P], fp8, tag="oh")
                    nc.sync.dma_start(
                        out=x2[:, 0:kk, :],
                        in_=x_d[j * P:(j + kk) * P, :].rearrange(
                            "(k p) c -> p k c", p=P))
                    nc.sync.dma_start(
                        out=oh2[:, 0:kk, :],
                        in_=ohb_d[j * P:(j + kk) * P, :].rearrange(
                            "(k p) c -> p k c", p=P))
                xt = x2[:, j % KB, :]
                oht = oh2[:, j % KB, :]

                # ACT: exp + row-sum accumulator
                e_scr = est.tile([P, C], bf16, tag="e")
                nc.scalar.activation(out=e_scr[:], in_=xt, func=AF.Exp,
                                     accum_out=s_sb[:, j:j + 1])

                # DVE: row max via half-fold, then winner mask (bf16 4x)
                h = work.tile([P, 500], bf16, tag="h")
                nc.vector.tensor_tensor(out=h[:], in0=xt[:, 0:500],
                                        in1=xt[:, 500:1000], op=ALU.max)
                m = work.tile([P, 1], f32, tag="m")
                nc.vector.reduce_max(out=m[:], in_=h[:],
                                     axis=mybir.AxisListType.X)
                wp = work.tile([P, C], bf16, tag="wp")
                nc.vector.tensor_scalar(out=wp[:], in0=xt, scalar1=m[:],
                                        scalar2=None, op0=ALU.is_equal)

                # PE: target-logit gather + histogram
                if first[j]:
                    cAB = psA.tile([P, 2, 512], f32, tag="cAB")
                    uP = psU.tile([P, P], f32, tag="uP")
                nc.tensor.matmul(out=uP[:, 0:ncls], lhsT=oht,
                                 rhs=xt[:, wlo:whi],
                                 start=bool(first[j]), stop=bool(last[j]))
                nc.tensor.matmul(out=cAB[:, 0, 0:500], lhsT=oht,
                                 rhs=wp[:, 0:500],
                                 start=bool(first[j]), stop=bool(last[j]))
                nc.tensor.matmul(out=cAB[:, 1, 0:500], lhsT=oht,
                                 rhs=wp[:, 500:1000],
                                 start=bool(first[j]), stop=bool(last[j]))

                if last[j]:
                    st = stg.tile([P, 2, 500], bf16, tag="st")
                    nc.scalar.activation(out=st[:], in_=cAB[:, :, 0:500],
                                         func=AF.Identity,
                                         accum_out=kacc[:, w:w + 1])
                    nc.scalar.copy(out=u_sb[:, w, 0:ncls], in_=uP[:, 0:ncls])
                    nc.sync.dma_start(
                        out=cdram[w][:].rearrange("p (k c) -> p k c", k=2),
                        in_=st[:])
                    nc.gpsimd.collective_compute(
                        "ReduceScatter", ALU.add, replica_groups=replica,
                        ins=[cdram[w][:].opt()], outs=[rsout[w][:].opt()])

            while True:
                if stage <= 1:
                    nc.sync.dma_start(out=loss_d, in_=s_sb[0:1, 0:1])
                    break

                # lse = Ln(sum exp); masked sum over valid samples
                nc.scalar.activation(out=lse_sb[:], in_=s_sb[:], func=AF.Ln)
                lsum = ph2.tile([P, 1], f32)
                lse_junk = ph2.tile([P, T], f32)
                nc.vector.scalar_tensor_tensor(
                    out=lse_junk[:], in0=lse_sb[:], scalar=1.0,
                    in1=valid_sb[:], op0=ALU.mult, op1=ALU.mult,
                    accum_out=lsum[:])

                # u diagonal per window -> sum
                diag_junk = ph2.tile([P, P], f32)
                for w in range(NW):
                    nc.vector.scalar_tensor_tensor(
                        out=diag_junk[:], in0=u_sb[:, w, :], scalar=1.0,
                        in1=ident_sb[:], op0=ALU.mult, op1=ALU.mult,
                        accum_out=udiag_sb[:, w:w + 1])
                usum = ph2.tile([P, 1], f32)
                nc.vector.reduce_sum(out=usum[:], in_=udiag_sb[:],
                                     axis=mybir.AxisListType.X)
                ksum = ph2.tile([P, 1], f32)
                nc.vector.reduce_sum(out=ksum[:], in_=kacc[:],
                                     axis=mybir.AxisListType.X)

                if stage <= 2:
                    nc.sync.dma_start(out=loss_d, in_=usum[0:1, 0:1])
                    break

                # gather this core's RS strips: [16, 1000] x 8 -> [128, 1000]
                crs_sb = ph2.tile([P, C], bf16)
                for w in range(NW):
                    nc.sync.dma_start(out=crs_sb[SR * w:SR * (w + 1), :],
                                      in_=rsout[w][:])
                crs32 = ph2.tile([P, C], f32)
                nc.scalar.copy(out=crs32[:], in_=crs_sb[:])

                if stage <= 3:
                    nc.sync.dma_start(out=loss_d, in_=crs32[0:1, 0:1])
                    break

                # cm = counts + 1 + cost ; diag -> 1 via eye masks
                cm = ph2.tile([P, C], f32)
                nc.vector.scalar_tensor_tensor(out=cm[:], in0=crs32[:],
                                               scalar=1.0, in1=cost_sb[:],
                                               op0=ALU.add, op1=ALU.add)
                cm2 = ph2.tile([P, C], f32)
                nc.vector.tensor_tensor(out=cm2[:], in0=cm[:], in1=eyem_sb[:],
                                        op=ALU.mult)
                nc.vector.tensor_tensor(out=cm2[:], in0=cm2[:], in1=eyec_sb[:],
                                        op=ALU.add)

                # per-core partials: -mn, mx, S, usum, lsum, K
                pvec = ph2.tile([P, 8], f32)
                nc.vector.memset(pvec[:], 0.0)
                nc.vector.tensor_reduce(out=pvec[:, 0:1], in_=cm2[:],
                                        axis=mybir.AxisListType.X, op=ALU.min,
                                        negate=True)
                nc.vector.tensor_reduce(out=pvec[:, 1:2], in_=cm2[:],
                                        axis=mybir.AxisListType.X, op=ALU.max)
                nc.vector.scalar_tensor_tensor(
                    out=cm[:], in0=crs32[:], scalar=1.0, in1=cm2[:],
                    op0=ALU.mult, op1=ALU.mult, accum_out=pvec[:, 2:3])
                nc.vector.tensor_copy(out=pvec[:, 3:4], in_=usum[:])
                nc.vector.tensor_copy(out=pvec[:, 4:5], in_=lsum[:])
                nc.vector.tensor_copy(out=pvec[:, 5:6], in_=ksum[:])

                if stage <= 4:
                    nc.sync.dma_start(out=loss_d, in_=pvec[0:1, 0:1])
                    break

                # transpose partials -> rows; rows 0,1 combine via max,
                # rows 2-5 via sum (blend with 0/1 masks)
                tp = psT.tile([8, P], f32)
                nc.tensor.transpose(out=tp[:], in_=pvec[:], identity=ident_sb[:])
                tv = ph2.tile([8, P], f32)
                nc.scalar.copy(out=tv[:], in_=tp[:])

                def blended_reduce(dst, src, ncols):
                    rmax = ph2.tile([8, 1], f32, tag=f"rmax{ncols}")
                    radd = ph2.tile([8, 1], f32, tag=f"radd{ncols}")
                    nc.vector.tensor_reduce(out=rmax[:], in_=src,
                                            axis=mybir.AxisListType.X, op=ALU.max)
                    nc.vector.tensor_reduce(out=radd[:], in_=src,
                                            axis=mybir.AxisListType.X, op=ALU.add)
                    nc.vector.tensor_tensor(out=rmax[:], in0=rmax[:], in1=mA_sb[:],
                                            op=ALU.mult)
                    nc.vector.tensor_tensor(out=radd[:], in0=radd[:], in1=mB_sb[:],
                                            op=ALU.mult)
                    nc.vector.tensor_tensor(out=dst, in0=rmax[:], in1=radd[:],
                                            op=ALU.add)

                scal_col = ph2.tile([8, 1], f32)
                blended_reduce(scal_col[:], tv[:], P)

                if stage <= 5:
                    nc.sync.dma_start(out=loss_d, in_=scal_col[0:1, 0:1])
                    break

                # AllGather the 6 per-core scalars (padded to 8)
                scal_dram = dram.tile([8, 1], f32)
                nc.sync.dma_start(out=scal_dram[:], in_=scal_col[:])
                gath_dram = dram.tile([NCORE * 8, 1], f32)
                nc.gpsimd.collective_compute(
                    "AllGather", ALU.bypass, replica_groups=replica,
                    ins=[scal_dram[:].opt()], outs=[gath_dram[:].opt()])
                gt = ph2.tile([8, NCORE], f32)
                nc.sync.dma_start(
                    out=gt[:], in_=gath_dram[:].rearrange("(r k) c -> k (r c)",
                                                          k=8))
                scal2 = ph2.tile([8, 1], f32)
                blended_reduce(scal2[:], gt[:], NCORE)
                if stage <= 6:
                    nc.sync.dma_start(out=loss_d, in_=scal2[0:1, 0:1])
                    break

                # transpose [8,1] -> [1,8] on PE to land scalars on partition 0
                tp2 = psT.tile([1, 8], f32, tag="tp2")
                nc.tensor.transpose(out=tp2[:], in_=scal2[:],
                                    identity=ident_sb[0:8, 0:8])
                svec = ph2.tile([1, 8], f32)
                nc.scalar.copy(out=svec[:], in_=tp2[:])

                mnneg = svec[:, 0:1]   # -mn
                mx = svec[:, 1:2]
                St = svec[:, 2:3]
                Ut = svec[:, 3:4]
                Lt = svec[:, 4:5]
                Kt = svec[:, 5:6]

                glp = ph2.tile([1, 1], f32)
                nc.vector.tensor_tensor(out=glp[:], in0=Ut, in1=Lt,
                                        op=ALU.subtract)
                nc.vector.tensor_scalar(out=glp[:], in0=glp[:],
                                        scalar1=1.0 / b_tot, scalar2=None,
                                        op0=ALU.mult)
                rk = ph2.tile([1, 1], f32)
                nc.vector.reciprocal(out=rk[:], in_=Kt)
                d = ph2.tile([1, 1], f32)
                nc.vector.tensor_tensor(out=d[:], in0=mx, in1=mnneg,
                                        op=ALU.add)
                rd = ph2.tile([1, 1], f32)
                nc.vector.reciprocal(out=rd[:], in_=d[:])
                q = ph2.tile([1, 1], f32)
                nc.vector.tensor_tensor(out=q[:], in0=St, in1=rk[:],
                                        op=ALU.mult)
                nc.vector.tensor_tensor(out=q[:], in0=q[:], in1=mnneg,
                                        op=ALU.add)
                nc.vector.tensor_tensor(out=q[:], in0=q[:], in1=rd[:],
                                        op=ALU.mult)
                gc = ph2.tile([1, 1], f32)
                nc.vector.tensor_scalar(out=gc[:], in0=q[:],
                                        scalar1=BETA2 - BETA1, scalar2=BETA1,
                                        op0=ALU.mult, op1=ALU.add)
                loss = ph2.tile([1, 1], f32)
                nc.vector.scalar_tensor_tensor(out=loss[:], in0=glp[:],
                                               scalar=-1.0, in1=gc[:],
                                               op0=ALU.mult, op1=ALU.mult)
                nc.sync.dma_start(out=loss_d, in_=loss[:])
                break

    nc.compile()
    return nc


# ----------------------------------------------------------------------------
# Entry points
# ----------------------------------------------------------------------------

def _prepare(outputs, targets, cost_matrix):
    in_maps, win_of_tile, T = _build_inputs(outputs, targets, cost_matrix)
    b_tot = int(np.asarray(targets).shape[0])
    nc = _build_program(T, win_of_tile, b_tot)
    return nc, in_maps


def _install_ntff_hook():
    """Register the axon NTFF profiling hook that the agent image's antenv
    stub lacks (mirrors trn_agent_boot's _ntff_profile_via_ctypes)."""
    import sys
    import types
    import ctypes
    import contextlib
    try:
        from antenv.axon_hooks import get_axon_ntff_profile_hook  # noqa
        return True
    except ImportError:
        pass
    so_path = "/opt/axon/libaxon_pjrt.so"
    if not os.path.exists(so_path):
        return False
    lib = ctypes.CDLL(so_path)
    if not hasattr(lib, "axon_start_nrt_profile"):
        return False
    lib.axon_start_nrt_profile.argtypes = [ctypes.POINTER(ctypes.c_int64),
                                           ctypes.c_size_t]
    lib.axon_start_nrt_profile.restype = ctypes.c_int64
    lib.axon_stop_nrt_profile.argtypes = [ctypes.c_char_p]
    lib.axon_stop_nrt_profile.restype = ctypes.c_int64

    @contextlib.contextmanager
    def _hook(output_dir, device_ids):
        import jax
        jax.devices()
        if device_ids:
            ids = (ctypes.c_int64 * len(device_ids))(*device_ids)
            rc = lib.axon_start_nrt_profile(ids, len(device_ids))
        else:
            rc = lib.axon_start_nrt_profile(None, 0)
        if rc != 0:
            raise RuntimeError(f"axon_start_nrt_profile rc={rc}")
        try:
            yield
        finally:
            n = lib.axon_stop_nrt_profile(str(output_dir).encode())
            print(f"ntff profile: {n} file(s) -> {output_dir}")

    mod = types.ModuleType("antenv.axon_hooks")
    mod.get_axon_ntff_profile_hook = lambda: _hook
    mod.set_axon_ntff_profile_hook = lambda h: None
    sys.modules["antenv.axon_hooks"] = mod
    return True


def kernel(outputs, targets, cost_matrix):
    targets = np.asarray(targets)
    nc, in_maps = _prepare(outputs, targets, cost_matrix)
    from concourse.bass_utils import run_bass_kernel_spmd
    trace = os.environ.get("KERNEL_TRACE", "0") == "1"
    if trace:
        trace = _install_ntff_hook()
    res = run_bass_kernel_spmd(nc, in_maps, list(range(NCORE)), trace=trace,
                               tmpdir=os.environ.get("KERNEL_TRACE_DIR"))
    if trace and res.exec_time_ns is not None:
        print(f"HW exec time: {res.exec_time_ns} ns")
    loss = np.asarray(res.results[0]["loss"]).reshape(-1)[0]
    return np.float32(loss)


def kernel_sim(outputs, targets, cost_matrix):
    """CoreSim validation path (no hardware)."""
    import concourse.bass_interp as bass_interp
    nc, in_maps = _prepare(outputs, targets, cost_matrix)
    sim = bass_interp.MultiCoreSim(nc, num_cores=NCORE)
    for i in range(NCORE):
        for k, v in in_maps[i].items():
            sim.cores[i].tensor(k)[:] = v
    sim.simulate(check_with_hw=False)
    return np.float32(np.asarray(sim.cores[0].mem_tensor("loss")).reshape(-1)[0])


# revision 16
# speedup vs baseline: 1.1056x; 1.1056x over previous
"""Trainium2 Bass kernel for nn_CostSensitiveCrossEntropyLossN.

Reference semantics (B=131072 samples, C=1000 classes):
    log_probs = log_softmax(outputs)            # [B, C]
    predicted = argmax(outputs, axis=1)         # [B]
    cm = cost_matrix; cm[t_i, p_i] += 1 per sample
    cm = cm * (1 - eye) + 1;  mn = min(cm); mx = max(cm)
    cm = 1 + (cm - mn) / (mx - mn)
    loss = -mean_i(log_probs[i, t_i]) * mean_i(cm[t_i, p_i])

Key identities:
    sum_i cm[t_i, p_i] = sum_{a,b} counts[a,b] * cm[a,b]  (counts = histogram)
    x is streamed in bf16; the winner mask (x == rowmax) may then mark several
    tied columns per row, so the gather mean is normalized by K = sum(counts)
    instead of B (validated: rel err ~2e-4 on the reference data).

Distribution (8 NeuronCores, data-parallel over batch):
  Host deals samples round-robin per 128-class target window, pads each window
  to a uniform tile count so one SPMD program serves all cores, and uploads
  per-tile target one-hots (derived from the integer labels) plus bf16 inputs.

Per 128-sample tile on device:
  ACT: exp(x) with fused row-sum accumulation  -> lse later via Ln
  DVE: row max (half-fold + reduce); winner mask wp = (x == rowmax) in bf16 4x
  PE:  uP[w] += onehot_t^T @ x[:, window]  (target logits)
       counts[w] += onehot_t^T @ wp        (histogram rides the PE)
Tail: per-window ReduceScatter kicked as each window's histogram completes
  (overlaps the main loop), per-core slice of cm normalize, one AllGather of
  6 scalars, final scalar math.
"""
import os
import numpy as np
import ml_dtypes

NCORE = 8
P = 128
C = 1000
NW = 8              # class windows (classes padded to NW*P = 1024)
SR = P // NCORE     # ReduceScatter strip rows per core per window (16)
BETA1, BETA2 = 1.0, 2.0


# ----------------------------------------------------------------------------
# Host-side prep
# ----------------------------------------------------------------------------

def _host_prep(targets):
    t = np.asarray(targets).astype(np.int64)
    tw_all = t // P
    per_cw = [[None] * NW for _ in range(NCORE)]
    for w in range(NW):
        sel = np.where(tw_all == w)[0]
        sel = sel[np.argsort(t[sel], kind="stable")]
        for c in range(NCORE):
            per_cw[c][w] = sel[c::NCORE]
    T_w = []
    for w in range(NW):
        n_max = max(len(per_cw[c][w]) for c in range(NCORE))
        T_w.append(max(1, -(-n_max // P)))
    T = int(sum(T_w))
    rows = np.zeros((NCORE, T * P), dtype=np.int64)
    tloc = np.full((NCORE, T * P), -1, dtype=np.int64)
    valid = np.zeros((NCORE, T * P), dtype=np.float32)
    win_of_tile = np.concatenate(
        [np.full(T_w[w], w, dtype=np.int64) for w in range(NW)])
    for c in range(NCORE):
        off = 0
        for w in range(NW):
            sel = per_cw[c][w]
            n = len(sel)
            cap = T_w[w] * P
            rows[c, off:off + n] = sel
            rows[c, off + n:off + cap] = sel[0] if n > 0 else 0
            tloc[c, off:off + n] = t[sel] - P * w
            valid[c, off:off + n] = 1.0
            off += cap
    return rows, tloc, valid, win_of_tile, T


def _build_inputs(outputs, targets, cost_matrix):
    rows, tloc, valid, win_of_tile, T = _host_prep(targets)
    outputs = np.asarray(outputs, dtype=np.float32)
    cost = np.asarray(cost_matrix, dtype=np.float32)
    ident = np.eye(P, dtype=np.float32)
    mA = np.array([1, 1, 0, 0, 0, 0, 0, 0], dtype=np.float32)[:, None]
    mB = np.array([0, 0, 1, 1, 1, 1, 0, 0], dtype=np.float32)[:, None]
    ar = np.arange(T * P)
    in_maps = []
    for c in range(NCORE):
        x_c = outputs[rows[c]].astype(ml_dtypes.bfloat16)       # [T*P, C]
        ohb = np.zeros((T * P, P), dtype=ml_dtypes.float8_e4m3)
        real = tloc[c] >= 0
        ohb[ar[real], tloc[c][real]] = 1.0
        # this core's post-RS classes: for each window w, rows 16c..16c+15
        cls = (128 * np.repeat(np.arange(NW), SR)
               + SR * c + np.tile(np.arange(SR), NW))            # [P]
        cost_slab = np.full((P, C), 5.0, dtype=np.float32)
        eyec = np.zeros((P, C), dtype=np.float32)
        ok = cls < C
        cost_slab[ok] = cost[cls[ok]]
        eyec[np.arange(P)[ok], cls[ok]] = 1.0
        in_maps.append({
            "x": x_c,
            "ohb": ohb,
            "valid": np.ascontiguousarray(valid[c].reshape(T, P).T),
            "cost": cost_slab,
            "ident": ident,
            "eyec": eyec,
            "eyem": 1.0 - eyec,
            "mA": mA,
            "mB": mB,
        })
    return in_maps, win_of_tile, T


# ----------------------------------------------------------------------------
# Device program
# ----------------------------------------------------------------------------

def _build_program(T, win_of_tile, b_tot, stage=99):
    import concourse.bacc as bacc
    import concourse.tile as tile
    import concourse.mybir as mybir

    f32 = mybir.dt.float32
    bf16 = mybir.dt.bfloat16
    fp8 = mybir.dt.float8e4
    ALU = mybir.AluOpType
    AF = mybir.ActivationFunctionType

    nc = bacc.Bacc("TRN2", target_bir_lowering=False, debug=False,
                   num_devices=NCORE)

    x_d = nc.dram_tensor("x", [T * P, C], bf16, kind="ExternalInput").ap()
    ohb_d = nc.dram_tensor("ohb", [T * P, P], fp8, kind="ExternalInput").ap()
    valid_d = nc.dram_tensor("valid", [P, T], f32, kind="ExternalInput").ap()
    cost_d = nc.dram_tensor("cost", [P, C], f32, kind="ExternalInput").ap()
    ident_d = nc.dram_tensor("ident", [P, P], f32, kind="ExternalInput").ap()
    eyec_d = nc.dram_tensor("eyec", [P, C], f32, kind="ExternalInput").ap()
    eyem_d = nc.dram_tensor("eyem", [P, C], f32, kind="ExternalInput").ap()
    mA_d = nc.dram_tensor("mA", [8, 1], f32, kind="ExternalInput").ap()
    mB_d = nc.dram_tensor("mB", [8, 1], f32, kind="ExternalInput").ap()
    loss_d = nc.dram_tensor("loss", [1, 1], f32, kind="ExternalOutput").ap()

    first = np.zeros(T, dtype=bool)
    last = np.zeros(T, dtype=bool)
    for j in range(T):
        w = win_of_tile[j]
        first[j] = (j == 0) or (win_of_tile[j - 1] != w)
        last[j] = (j == T - 1) or (win_of_tile[j + 1] != w)

    replica = [list(range(NCORE))]
    KB = 4  # tiles per input DMA batch

    with tile.TileContext(nc) as tc:
        with (
            tc.tile_pool(name="io", bufs=1) as io,
            tc.tile_pool(name="xs", bufs=3) as xs,
            tc.tile_pool(name="work", bufs=4) as work,
            tc.tile_pool(name="est", bufs=3) as est,
            tc.tile_pool(name="stg", bufs=2) as stg,
            tc.tile_pool(name="accum", bufs=1) as acc,
            tc.tile_pool(name="ph2", bufs=1) as ph2,
            tc.tile_pool(name="psA", bufs=2, space="PSUM") as psA,
            tc.tile_pool(name="psU", bufs=2, space="PSUM") as psU,
            tc.tile_pool(name="psT", bufs=1, space="PSUM") as psT,
            tc.tile_pool(name="dram", bufs=1, space="DRAM") as dram,
        ):
            # persistent inputs
            valid_sb = io.tile([P, T], f32)
            cost_sb = io.tile([P, C], f32)
            ident_sb = io.tile([P, P], f32)
            eyec_sb = io.tile([P, C], f32)
            eyem_sb = io.tile([P, C], f32)
            mA_sb = io.tile([8, 1], f32)
            mB_sb = io.tile([8, 1], f32)
            for sb, d in ((valid_sb, valid_d), (cost_sb, cost_d),
                          (ident_sb, ident_d), (eyec_sb, eyec_d),
                          (eyem_sb, eyem_d), (mA_sb, mA_d), (mB_sb, mB_d)):
                nc.sync.dma_start(out=sb[:], in_=d)

            # persistent accumulators
            s_sb = acc.tile([P, T], f32)          # row sum(exp)
            lse_sb = acc.tile([P, T], f32)
            u_sb = acc.tile([P, NW, P], f32)
            kacc = acc.tile([P, NW], f32)
            udiag_sb = acc.tile([P, NW], f32)
            nc.vector.memset(u_sb[:], 0.0)

            # per-window collective buffers
            cdram = [dram.tile([P, C], bf16, tag=f"cd{w}", name=f"cd{w}")
                     for w in range(NW)]
            rsout = [dram.tile([SR, C], bf16, tag=f"ro{w}", name=f"ro{w}")
                     for w in range(NW)]

            cA = cB = uP = None
            x2 = oh2 = None
            for j in range(T):
                w = int(win_of_tile[j])
                wlo = w * P
                whi = min(C, wlo + P)
                ncls = whi - wlo

                if j % KB == 0:
                    kk = min(KB, T - j)
                    x2 = xs.tile([P, KB, C], bf16, tag="x")
                    oh2 = xs.tile([P, KB, P], fp8, tag="oh")
                    nc.sync.dma_start(
                        out=x2[:, 0:kk, :],
                        in_=x_d[j * P:(j + kk) * P, :].rearrange(
                            "(k p) c -> p k c", p=P))
                    nc.sync.dma_start(
                        out=oh2[:, 0:kk, :],
                        in_=ohb_d[j * P:(j + kk) * P, :].rearrange(
                            "(k p) c -> p k c", p=P))
                xt = x2[:, j % KB, :]
                oht = oh2[:, j % KB, :]

                # ACT: exp + row-sum accumulator
                e_scr = est.tile([P, C], bf16, tag="e")
                nc.scalar.activation(out=e_scr[:], in_=xt, func=AF.Exp,
                                     accum_out=s_sb[:, j:j + 1])

                # DVE: row max via half-fold, then winner mask (bf16 4x)
                h = work.tile([P, 500], bf16, tag="h")
                nc.vector.tensor_tensor(out=h[:], in0=xt[:, 0:500],
                                        in1=xt[:, 500:1000], op=ALU.max)
                m = work.tile([P, 1], f32, tag="m")
                nc.vector.reduce_max(out=m[:], in_=h[:],
                                     axis=mybir.AxisListType.X)
                wp = work.tile([P, C], bf16, tag="wp")
                nc.vector.tensor_scalar(out=wp[:], in0=xt, scalar1=m[:],
                                        scalar2=None, op0=ALU.is_equal)

                # PE: target-logit gather + histogram
                if first[j]:
                    cAB = psA.tile([P, 2, 512], f32, tag="cAB")
                    uP = psU.tile([P, P], f32, tag="uP")
                nc.tensor.matmul(out=uP[:, 0:ncls], lhsT=oht,
                                 rhs=xt[:, wlo:whi],
                                 start=bool(first[j]), stop=bool(last[j]))
                nc.tensor.matmul(out=cAB[:, 0, 0:500], lhsT=oht,
                                 rhs=wp[:, 0:500],
                                 start=bool(first[j]), stop=bool(last[j]))
                nc.tensor.matmul(out=cAB[:, 1, 0:500], lhsT=oht,
                                 rhs=wp[:, 500:1000],
                                 start=bool(first[j]), stop=bool(last[j]))

                if last[j]:
                    st = stg.tile([P, 2, 500], bf16, tag="st")
                    nc.scalar.activation(out=st[:], in_=cAB[:, :, 0:500],
                                         func=AF.Identity,
                                         accum_out=kacc[:, w:w + 1])
                    nc.scalar.copy(out=u_sb[:, w, 0:ncls], in_=uP[:, 0:ncls])
                    nc.sync.dma_start(
                        out=cdram[w][:].rearrange("p (k c) -> p k c", k=2),
                        in_=st[:])
                    nc.gpsimd.collective_compute(
                        "ReduceScatter", ALU.add, replica_groups=replica,
                        ins=[cdram[w][:].opt()], outs=[rsout[w][:].opt()])

            while True:
                if stage <= 1:
                    nc.sync.dma_start(out=loss_d, in_=s_sb[0:1, 0:1])
                    break

                # lse = Ln(sum exp); masked sum over valid samples
                nc.scalar.activation(out=lse_sb[:], in_=s_sb[:], func=AF.Ln)
                lsum = ph2.tile([P, 1], f32)
                lse_junk = ph2.tile([P, T], f32)
                nc.vector.scalar_tensor_tensor(
                    out=lse_junk[:], in0=lse_sb[:], scalar=1.0,
                    in1=valid_sb[:], op0=ALU.mult, op1=ALU.mult,
                    accum_out=lsum[:])

                # u diagonal per window -> sum
                diag_junk = ph2.tile([P, P], f32)
                for w in range(NW):
                    nc.vector.scalar_tensor_tensor(
                        out=diag_junk[:], in0=u_sb[:, w, :], scalar=1.0,
                        in1=ident_sb[:], op0=ALU.mult, op1=ALU.mult,
                        accum_out=udiag_sb[:, w:w + 1])
                usum = ph2.tile([P, 1], f32)
                nc.vector.reduce_sum(out=usum[:], in_=udiag_sb[:],
                                     axis=mybir.AxisListType.X)
                ksum = ph2.tile([P, 1], f32)
                nc.vector.reduce_sum(out=ksum[:], in_=kacc[:],
                                     axis=mybir.AxisListType.X)

                if stage <= 2:
                    nc.sync.dma_start(out=loss_d, in_=usum[0:1, 0:1])
                    break

                # gather this core's RS strips: [16, 1000] x 8 -> [128, 1000]
                crs_sb = ph2.tile([P, C], bf16)
                for w in range(NW):
                    nc.sync.dma_start(out=crs_sb[SR * w:SR * (w + 1), :],
                                      in_=rsout[w][:])
                crs32 = ph2.tile([P, C], f32)
                nc.scalar.copy(out=crs32[:], in_=crs_sb[:])

                if stage <= 3:
                    nc.sync.dma_start(out=loss_d, in_=crs32[0:1, 0:1])
                    break

                # cm = counts + 1 + cost ; diag -> 1 via eye masks
                cm = ph2.tile([P, C], f32)
                nc.vector.scalar_tensor_tensor(out=cm[:], in0=crs32[:],
                                               scalar=1.0, in1=cost_sb[:],
                                               op0=ALU.add, op1=ALU.add)
                cm2 = ph2.tile([P, C], f32)
                nc.vector.tensor_tensor(out=cm2[:], in0=cm[:], in1=eyem_sb[:],
                                        op=ALU.mult)
                nc.vector.tensor_tensor(out=cm2[:], in0=cm2[:], in1=eyec_sb[:],
                                        op=ALU.add)

                # per-core partials: -mn, mx, S, usum, lsum, K
                pvec = ph2.tile([P, 8], f32)
                nc.vector.memset(pvec[:], 0.0)
                nc.vector.tensor_reduce(out=pvec[:, 0:1], in_=cm2[:],
                                        axis=mybir.AxisListType.X, op=ALU.min,
                                        negate=True)
                nc.vector.tensor_reduce(out=pvec[:, 1:2], in_=cm2[:],
                                        axis=mybir.AxisListType.X, op=ALU.max)
                nc.vector.scalar_tensor_tensor(
                    out=cm[:], in0=crs32[:], scalar=1.0, in1=cm2[:],
                    op0=ALU.mult, op1=ALU.mult, accum_out=pvec[:, 2:3])
                nc.vector.tensor_copy(out=pvec[:, 3:4], in_=usum[:])
                nc.vector.tensor_copy(out=pvec[:, 4:5], in_=lsum[:])
                nc.vector.tensor_copy(out=pvec[:, 5:6], in_=ksum[:])

                if stage <= 4:
                    nc.sync.dma_start(out=loss_d, in_=pvec[0:1, 0:1])
                    break

                # transpose partials -> rows; rows 0,1 combine via max,
                # rows 2-5 via sum (blend with 0/1 masks)
                tp = psT.tile([8, P], f32)
                nc.tensor.transpose(out=tp[:], in_=pvec[:], identity=ident_sb[:])
                tv = ph2.tile([8, P], f32)
                nc.scalar.copy(out=tv[:], in_=tp[:])

                def blended_reduce(dst, src, ncols):
                    rmax = ph2.tile([8, 1], f32, tag=f"rmax{ncols}")
                    radd = ph2.tile([8, 1], f32, tag=f"radd{ncols}")
                    nc.vector.tensor_reduce(out=rmax[:], in_=src,
                                            axis=mybir.AxisListType.X, op=ALU.max)
                    nc.vector.tensor_reduce(out=radd[:], in_=src,
                                            axis=mybir.AxisListType.X, op=ALU.add)
                    nc.vector.tensor_tensor(out=rmax[:], in0=rmax[:], in1=mA_sb[:],
                                            op=ALU.mult)
                    nc.vector.tensor_tensor(out=radd[:], in0=radd[:], in1=mB_sb[:],
                                            op=ALU.mult)
                    nc.vector.tensor_tensor(out=dst, in0=rmax[:], in1=radd[:],
                                            op=ALU.add)

                scal_col = ph2.tile([8, 1], f32)
                blended_reduce(scal_col[:], tv[:], P)

                if stage <= 5:
                    nc.sync.dma_start(out=loss_d, in_=scal_col[0:1, 0:1])
                    break

                # AllGather the 6 per-core scalars (padded to 8)
                scal_dram = dram.tile([8, 1], f32)
                nc.sync.dma_start(out=scal_dram[:], in_=scal_col[:])
                gath_dram = dram.tile([NCORE * 8, 1], f32)
                nc.gpsimd.collective_compute(
                    "AllGather", ALU.bypass, replica_groups=replica,
                    ins=[scal_dram[:].opt()], outs=[gath_dram[:].opt()])
                gt = ph2.tile([8, NCORE], f32)
                nc.sync.dma_start(
                    out=gt[:], in_=gath_dram[:].rearrange("(r k) c -> k (r c)",
                                                          k=8))
                scal2 = ph2.tile([8, 1], f32)
                blended_reduce(scal2[:], gt[:], NCORE)
                if stage <= 6:
                    nc.sync.dma_start(out=loss_d, in_=scal2[0:1, 0:1])
                    break

                # transpose [8,1] -> [1,8] on PE to land scalars on partition 0
                tp2 = psT.tile([1, 8], f32, tag="tp2")
                nc.tensor.transpose(out=tp2[:], in_=scal2[:],
                                    identity=ident_sb[0:8, 0:8])
                svec = ph2.tile([1, 8], f32)
                nc.scalar.copy(out=svec[:], in_=tp2[:])

                mnneg = svec[:, 0:1]   # -mn
                mx = svec[:, 1:2]
                St = svec[:, 2:3]
                Ut = svec[:, 3:4]
                Lt = svec[:, 4:5]
                Kt = svec[:, 5:6]

                glp = ph2.tile([1, 1], f32)
                nc.vector.tensor_tensor(out=glp[:], in0=Ut, in1=Lt,
                                        op=ALU.subtract)
                nc.vector.tensor_scalar(out=glp[:], in0=glp[:],
                                        scalar1=1.0 / b_tot, scalar2=None,
                                        op0=ALU.mult)
                rk = ph2.tile([1, 1], f32)
                nc.vector.reciprocal(out=rk[:], in_=Kt)
                d = ph2.tile([1, 1], f32)
                nc.vector.tensor_tensor(out=d[:], in0=mx, in1=mnneg,
                                        op=ALU.add)
                rd = ph2.tile([1, 1], f32)
                nc.vector.reciprocal(out=rd[:], in_=d[:])
                q = ph2.tile([1, 1], f32)
                nc.vector.tensor_tensor(out=q[:], in0=St, in1=rk[:],
                                        op=ALU.mult)
                nc.vector.tensor_tensor(out=q[:], in0=q[:], in1=mnneg,
                                        op=ALU.add)
                nc.vector.tensor_tensor(out=q[:], in0=q[:], in1=rd[:],
                                        op=ALU.mult)
                gc = ph2.tile([1, 1], f32)
                nc.vector.tensor_scalar(out=gc[:], in0=q[:],
                                        scalar1=BETA2 - BETA1, scalar2=BETA1,
                                        op0=ALU.mult, op1=ALU.add)
                loss = ph2.tile([1, 1], f32)
                nc.vector.scalar_tensor_tensor(out=loss[:], in0=glp[:],
                                               scalar=-1.0, in1=gc[:],
                                               op0=ALU.mult, op1=ALU.mult)
                nc.sync.dma_start(out=loss_d, in_=loss[:])
                break

    nc.compile()
    return nc


# ----------------------------------------------------------------------------
# Entry points
# ----------------------------------------------------------------------------

def _prepare(outputs, targets, cost_matrix):
    in_maps, win_of_tile, T = _build_inputs(outputs, targets, cost_matrix)
    b_tot = int(np.asarray(targets).shape[0])
    nc = _build_program(T, win_of_tile, b_tot)
    return nc, in_maps


def _install_ntff_hook():
    """Register the axon NTFF profiling hook that the agent image's antenv
    stub lacks (mirrors trn_agent_boot's _ntff_profile_via_ctypes)."""
    import sys
    import types
    import ctypes
    import contextlib
    try:
        from antenv.axon_hooks import get_axon_ntff_profile_hook  # noqa
        return True
    except ImportError:
        pass
    so_path = "/opt/axon/libaxon_pjrt.so"
    if not os.path.exists(so_path):
        return False
    lib = ctypes.CDLL(so_path)
    if not hasattr(lib, "axon_start_nrt_profile"):
        return False
    lib.axon_start_nrt_profile.argtypes = [ctypes.POINTER(ctypes.c_int64),
                                           ctypes.c_size_t]
    lib.axon_start_nrt_profile.restype = ctypes.c_int64
    lib.axon_stop_nrt_profile.argtypes = [ctypes.c_char_p]
    lib.axon_stop_nrt_profile.restype = ctypes.c_int64

    @contextlib.contextmanager
    def _hook(output_dir, device_ids):
        import jax
        jax.devices()
        if device_ids:
            ids = (ctypes.c_int64 * len(device_ids))(*device_ids)
            rc = lib.axon_start_nrt_profile(ids, len(device_ids))
        else:
            rc = lib.axon_start_nrt_profile(None, 0)
        if rc != 0:
            raise RuntimeError(f"axon_start_nrt_profile rc={rc}")
        try:
            yield
        finally:
            n = lib.axon_stop_nrt_profile(str(output_dir).encode())
            print(f"ntff profile: {n} file(s) -> {output_dir}")

    mod = types.ModuleType("antenv.axon_hooks")
    mod.get_axon_ntff_profile_hook = lambda: _hook
    mod.set_axon_ntff_profile_hook = lambda h: None
    sys.modules["antenv.axon_hooks"] = mod
    return True


def kernel(outputs, targets, cost_matrix):
    targets = np.asarray(targets)
    nc, in_maps = _prepare(outputs, targets, cost_matrix)
    from concourse.bass_utils import run_bass_kernel_spmd
    trace = os.environ.get("KERNEL_TRACE", "0") == "1"
    if trace:
        trace = _install_ntff_hook()
    res = run_bass_kernel_spmd(nc, in_maps, list(range(NCORE)), trace=trace,
                               tmpdir=os.environ.get("KERNEL_TRACE_DIR"))
    if trace and res.exec_time_ns is not None:
        print(f"HW exec time: {res.exec_time_ns} ns")
    loss = np.asarray(res.results[0]["loss"]).reshape(-1)[0]
    return np.float32(loss)


def kernel_sim(outputs, targets, cost_matrix):
    """CoreSim validation path (no hardware)."""
    import concourse.bass_interp as bass_interp
    nc, in_maps = _prepare(outputs, targets, cost_matrix)
    sim = bass_interp.MultiCoreSim(nc, num_cores=NCORE)
    for i in range(NCORE):
        for k, v in in_maps[i].items():
            sim.cores[i].tensor(k)[:] = v
    sim.simulate(check_with_hw=False)
    return np.float32(np.asarray(sim.cores[0].mem_tensor("loss")).reshape(-1)[0])
